# revision 33
# baseline (speedup 1.0000x reference)
"""Trainium2 Bass kernel for nn_BlockDiagonalLinearAlignment.

Math: y = x @ A, where A is a 128x128 block-diagonal matrix assembled from
dense / diagonal / low-rank 16x16 blocks, followed by row-wise L2
normalization: out = y / (||y||_2 + 1e-8).

Strategy (pure data parallel over the batch axis, 8 cores):
  - rel-err budget is 2e-2 -> compute in bf16 (measured rel err ~2.9e-3).
    Host casts x to bf16 AND pre-transposes each core shard to
    feature-major xT [128, 32768], so the kernel needs no PE transpose and
    input HBM traffic halves (16.8MB/core total vs 32MB in f32).
  - per chunk (16 tiles of 128 rows, two 2-bank PSUM groups, ps bufs=4 for
    fine-grained PSUM recycling): PE matmuls put y row-major into PSUM
    f32; a custom SEGMENTED DVE scan (SQSEG_ANT: prefix sum of squares
    whose accumulator resets at every 128-elem page via a patched
    SUB_DIM_DONE step state) reads PSUM directly (1-port op -> immune to
    the DVE/GPSIMD shared-SBUF-port contention) and makes pf[:, t, D-1]
    the tile's n2 with NO differencing op; a directly-emitted ACT Rsqrt
    per group reads those boundary values at stride 128.  Group 1 is
    scanned FIRST so its rsqrt finishes during the group-0 scan and the
    DVE scale op below never stalls the DVE queue.
  - scale split by measured per-op costs (DVE TT 1x ~0.14us/tile in
    multi-tile ops; ACT scale-copy ~0.46us/tile; GPSIMD ~2.1us dispatch +
    ~0.24us/tile up to a ~12-tile knee, beyond which big ops degrade
    sharply): GPSIMD takes KT=11 tiles via ONE big op on an f32 SBUF copy
    (ACT makes the copy; f32 in0 is ~20% faster than bf16 on GPSIMD),
    DVE takes DT=3 tiles via one TT-broadcast op from PSUM group 1, ACT
    takes AT=2 tiles from PSUM.  Group 0 holds only GPSIMD tiles so it
    frees right after scan+copy and the PE runs ahead.
  - out DMA per chunk (bf16), delayed OUT_DELAY chunks in the sync-ring
    FIFO so input prefetch is never queued behind an output's semaphore
    wait; host reorders back to row-major and upcasts to f32.
  - journey: 127.4us (f32) -> 78.8us (previous session's 3-pass split) ->
    77.6 -> 76.8us (this session: segmented scan from PSUM kills the
    memset+boundary-sub and the ACT bulk copy shrinks to the GPSIMD tiles
    only; scales read PSUM; g1-first scan order; delayed out-DMAs).
    Measured cliffs baked into the constants: GPSIMD+DVE share one SBUF
    port (2-port DVE SBUF ops run 2-6x slower under concurrent GPSIMD;
    PSUM-source DVE ops are immune); GPSIMD ops >12 tiles hit a steep
    knee (64-tile op = 13.9us); ACT accum_out costs a separate ~285ns
    ACTIVATION_READ_ACCUMULATOR; buffer counts interact chaotically with
    the Tile scheduler (inpool=10 is a measured groove; 8/11/12 all
    regress to 90+us).
"""

import contextlib
import functools
import sys

for _p in ("/opt/trn_rl_repo",):
    if _p not in sys.path:
        sys.path.append(_p)

import numpy as np
import ml_dtypes

import concourse.bacc as bacc
import concourse.bass as bass
import concourse.tile as tile
from concourse import bass_utils, mybir


def _register_sqscan():
    """Register a custom DVE op: out[p, k] = sum_{j<=k} in0[p, j]^2
    (inclusive prefix sum of squares along the free dim). Per-tile sums of
    squares are then recovered by differencing at tile boundaries, fusing
    what would otherwise be a tensor_tensor square + a tensor_reduce into
    one 1x DVE pass."""
    import re
    from concourse import dve_ops
    from concourse.dve_spec import Spec, Src0, sq, scan, AluOp
    from concourse.dve_table_gen import dve_ver_for

    name = "SQSCAN_ANT"
    for op in dve_ops.OPS:
        if op.name == name:
            return op
    spec = Spec(body=scan(AluOp.ADD, sq(Src0)))
    ver = dve_ver_for("TRN2")
    op = dve_ops.DveOp(name, spec, subdim=False, uops_sha={})
    dve_ops.OPS.append(op)
    dve_ops.CUSTOM_DVE_SPECS[name] = spec
    dve_ops._SUB_OPCODE_FOR_NAME[name] = (
        dve_ops._CUSTOM_DVE_ROW_BASE + len(dve_ops.OPS) - 1
    )
    try:
        op.compile(ver)
    except ValueError as e:
        m = re.search(r'="([0-9a-f]+)"', str(e))
        if m is None:
            raise
        op = dve_ops.DveOp(name, spec, subdim=False,
                           uops_sha={ver: m.group(1)})
        dve_ops.OPS[-1] = op
        dve_ops.CUSTOM_DVE_SPECS[name] = spec
    return op


SQSCAN = _register_sqscan()


def _register_sqscan_seg():
    """Register SQSEG_ANT: a SEGMENTED inclusive prefix-sum-of-squares. With
    in0/out shaped [P, S, N], the accumulator resets at every page boundary
    (acc <- 0 + in0^2 on the boundary element), so out[p, s, N-1] is the sum
    of squares of page s directly — no tile-boundary differencing needed.

    The stock Spec machinery only emits `acc <- op(acc, leaf)` step states
    (PageIdx); the reset form `acc <- op(0, expr)` is the same _Stage shape
    with the CURR operand swapped for Zero, so we patch the two tiny codegen
    helpers for the duration of this op's compile."""
    import re
    import concourse.dve_spec as ds
    from concourse import dve_ops
    from concourse.dve_spec import Spec, Src0, sq, AluOp
    from concourse.dve_table_gen import dve_ver_for

    name = "SQSEG_ANT"
    for op in dve_ops.OPS:
        if op.name == name:
            return op

    seg = ds.Scan(AluOp.ADD, sq(Src0), None)
    object.__setattr__(seg, "_subdim_step", ds.Zero)   # frozen dataclass
    object.__setattr__(seg, "_seg_reset", True)

    orig_nas = ds._node_as_stage
    orig_so = ds._scan_overrides

    def nas(e):
        if getattr(e, "_seg_reset", False):
            # steady stage: normal combine (not PageIdx's hold/BYPASS)
            return ds._Stage(e.op, ds.AluInp.CURR_ALU_OUT, e.expr)
        return orig_nas(e)

    def so(scans, node_stage):
        seed, step = orig_so(scans, node_stage)
        for sc in scans:
            if getattr(sc, "_seg_reset", False):
                # boundary element: acc <- 0 + expr(elem)
                step[node_stage[sc]] = ds._Stage(sc.op, ds.Zero, sc.expr)
        return seed, step

    spec = Spec(body=seg)
    ver = dve_ver_for("TRN2")
    op = dve_ops.DveOp(name, spec, subdim=True, uops_sha={})
    dve_ops.OPS.append(op)
    dve_ops.CUSTOM_DVE_SPECS[name] = spec
    dve_ops._SUB_OPCODE_FOR_NAME[name] = (
        dve_ops._CUSTOM_DVE_ROW_BASE + len(dve_ops.OPS) - 1
    )
    ds._node_as_stage, ds._scan_overrides = nas, so
    try:
        try:
            op.compile(ver)
        except ValueError as e:
            m = re.search(r'="([0-9a-f]+)"', str(e))
            if m is None:
                raise
            op = dve_ops.DveOp(name, spec, subdim=True,
                               uops_sha={ver: m.group(1)})
            dve_ops.OPS[-1] = op
            dve_ops.CUSTOM_DVE_SPECS[name] = spec
            op.compile(ver)
    finally:
        ds._node_as_stage, ds._scan_overrides = orig_nas, orig_so
    return op


SQSEG = _register_sqscan_seg()

B = 262144
D = 128
BS = 16
K = 8
N_CORES = 8
ROWS_PER_CORE = B // N_CORES  # 32768

DENSE = (0, 3, 6)
DIAG = (1, 4, 7)
LR = (2, 5)

F32 = mybir.dt.float32
BF16 = mybir.dt.bfloat16
NP_BF16 = ml_dtypes.bfloat16

P = 128
CHUNK = 2048            # rows per DMA chunk (per core)
DVE_TILES = 3           # tiles scaled by the DVE TT-broadcast op (PSUM)
ACT_TILES = 2           # tiles scaled by ACT per-tile (PSUM)
OUT_DELAY = 3           # chunks an out-DMA trails in the sync FIFO
BUFS = dict(inpool=10, outpool=6, ypool=4, pfpool=4, smalls=16, ps=4)
MULT = mybir.AluOpType.mult
ADD = mybir.AluOpType.add


def _assemble_A(W_dense, s_diag, U, V):
    """Full 128x128 block-diagonal transform, y = x @ A."""
    A = np.zeros((D, D), dtype=np.float32)
    for i, k in enumerate(DENSE):
        A[k * BS:(k + 1) * BS, k * BS:(k + 1) * BS] = W_dense[i].T
    for i, k in enumerate(DIAG):
        A[k * BS:(k + 1) * BS, k * BS:(k + 1) * BS] = np.diag(s_diag[i])
    for i, k in enumerate(LR):
        A[k * BS:(k + 1) * BS, k * BS:(k + 1) * BS] = V[i] @ U[i].T
    return A


def _scalar_rsqrt(nc, out, in_):
    """ACT Rsqrt, emitted directly (bass's activation() refuses Rsqrt for
    accuracy reasons; at a 2e-2 rel-err budget and n2 in [~50, 250] the
    table accuracy is more than sufficient). Mirrors activation() lowering:
    ins = [in_, bias(AP), scale(imm), alpha(imm)]."""
    se = nc.scalar
    bias_ap = nc.const_aps.scalar_like(0.0, in_)
    ins = [
        se.lower_ap(in_),
        se.lower_ap(bias_ap),
        mybir.ImmediateValue(dtype=mybir.dt.float32, value=1.0),
        mybir.ImmediateValue(dtype=mybir.dt.float32, value=0.0),
    ]
    return se.add_instruction(
        mybir.InstActivation(
            name=nc.get_next_instruction_name(),
            func=mybir.ActivationFunctionType.Rsqrt,
            ins=ins,
            outs=[se.lower_ap(out)],
        )
    )


def _kernel_body(ctx, tc, out_ap, xT_ap, amat_ap, rows, chunk):
    nc = tc.nc
    T = chunk // P                 # tiles per chunk (16)
    nchunks = rows // chunk
    assert rows % chunk == 0

    consts = ctx.enter_context(tc.tile_pool(name="consts", bufs=1))
    amat = consts.tile([P, P], BF16)
    nc.sync.dma_start(out=amat, in_=amat_ap)

    inpool = ctx.enter_context(tc.tile_pool(name="inpool", bufs=BUFS["inpool"]))
    outpool = ctx.enter_context(tc.tile_pool(name="outpool", bufs=BUFS["outpool"]))
    ypool = ctx.enter_context(tc.tile_pool(name="ypool", bufs=BUFS["ypool"]))
    pfpool = ctx.enter_context(tc.tile_pool(name="pfpool", bufs=BUFS["pfpool"]))
    smalls = ctx.enter_context(tc.tile_pool(name="smalls", bufs=BUFS["smalls"]))
    ps = ctx.enter_context(tc.tile_pool(name="ps", bufs=BUFS["ps"], space="PSUM"))

    DT, AT = DVE_TILES, ACT_TILES          # tiles scaled on DVE / ACT
    KT = T - DT - AT                       # tiles scaled on GPSIMD (via copy)
    assert KT >= T // 2 and DT + AT <= T // 2
    pending = []                           # delayed out-DMAs (sync-ring order:
                                           # keep input prefetch ahead of
                                           # output waits in the FIFO)
    for c in range(nchunks):
        in_sb = inpool.tile([P, chunk], BF16)
        nc.sync.dma_start(out=in_sb, in_=xT_ap[:, c * chunk:(c + 1) * chunk])
        if len(pending) > OUT_DELAY:
            ap, sb = pending.pop(0)
            nc.sync.dma_start(out=ap, in_=sb)

        # two 2-bank PSUM groups per chunk (bufs=4). Group 0 holds only
        # GPSIMD-copied tiles -> it frees right after scan+copy, so the PE
        # runs ahead. The DVE/ACT scale tiles live at the END of group 1.
        y_sb = ypool.tile([P, KT, D], F32)
        groups = []
        for q in range(2):
            y_ps = ps.tile([P, T // 2, D], F32)
            groups.append(y_ps)
            for t in range(T // 2):
                g = q * (T // 2) + t
                nc.tensor.matmul(
                    y_ps[:, t], lhsT=in_sb[:, g * P:(g + 1) * P],
                    rhs=amat, start=True, stop=True,
                )
            # ACT: copy this group's GPSIMD tiles (0..KT-1) to SBUF (f32)
            lo, hi = q * (T // 2), min(KT, (q + 1) * (T // 2))
            if lo < hi:
                nc.scalar.copy(y_sb[:, lo:hi], y_ps[:, 0:hi - lo])

        # DVE: SEGMENTED square-prefix-scan per PSUM group (1-port PSUM
        # reads: immune to GPSIMD port contention); accumulator resets every
        # tile, so pf[:, t, D-1] IS that tile's sum of squares. Group 1 is
        # scanned FIRST so its rsqrt completes during the group-0 scan and
        # the TT-scale below enters the DVE queue with zero wait.
        pf = pfpool.tile([P, T, D], F32)
        rp = smalls.tile([P, T], F32)
        nc.vector._custom_dve(SQSEG, out=pf[:, T // 2:T], in0=groups[1])
        # rsqrt split by CONSUMER: the DVE/ACT scale tiles' rsqrt fires
        # right after the g1 scan (so the TT never stalls the DVE queue);
        # the GPSIMD tiles' rsqrt comes after the g0 scan.
        _scalar_rsqrt(nc, rp[:, KT:T], pf[:, KT:T, D - 1])
        nc.vector._custom_dve(SQSEG, out=pf[:, 0:T // 2], in0=groups[0])
        _scalar_rsqrt(nc, rp[:, 0:KT], pf[:, 0:KT, D - 1])

        out_sb = outpool.tile([P, T, D], BF16)
        rb = rp.unsqueeze(2).broadcast_to([P, T, D])

        # scale split:
        #  - DVE: one TT-broadcast op from PSUM group 1 (PSUM port -> no
        #    GPSIMD contention; rsqrt_g1 already done -> no queue stall)
        if DT:
            nc.vector.tensor_mul(out_sb[:, KT:KT + DT],
                                 groups[1][:, KT - T // 2:KT - T // 2 + DT],
                                 rb[:, KT:KT + DT])
        #  - ACT: per-tile scale-copies from PSUM group 1
        for t in range(KT + DT, T):
            nc.scalar.activation(
                out_sb[:, t], groups[1][:, t - T // 2],
                mybir.ActivationFunctionType.Copy, scale=rp[:, t:t + 1],
            )
        #  - GPSIMD: one big op over its SBUF copy
        nc.gpsimd.tensor_mul(out_sb[:, 0:KT], y_sb, rb[:, 0:KT])

        pending.append((out_ap[c], out_sb))

    for ap, sb in pending:
        nc.sync.dma_start(out=ap, in_=sb)


@functools.lru_cache(maxsize=4)
def _build(rows, chunk):
    nc = bacc.Bacc(
        "TRN2",
        target_bir_lowering=False,
        debug=False,
        num_devices=1,
    )
    nchunks = rows // chunk
    T = chunk // P
    xT_t = nc.dram_tensor("xT", [P, rows], BF16, kind="ExternalInput").ap()
    a_t = nc.dram_tensor("amat", [D, D], BF16, kind="ExternalInput").ap()
    o_t = nc.dram_tensor("out", [nchunks, P, T * D], BF16,
                         kind="ExternalOutput").ap()
    with tile.TileContext(nc) as tc, contextlib.ExitStack() as ctx:
        _kernel_body(ctx, tc, o_t, xT_t, a_t, rows, chunk)
    nc.compile()
    return nc


def _run(x, A, trace=False, trace_cores=None):
    nc = _build(ROWS_PER_CORE, CHUNK)
    # host-side shard prep: per core, feature-major bf16 [128, ROWS_PER_CORE]
    xs = x.reshape(N_CORES, ROWS_PER_CORE, D).astype(NP_BF16)
    xTs = [np.ascontiguousarray(xs[i].T) for i in range(N_CORES)]
    A16 = A.astype(NP_BF16)
    in_maps = [{"xT": xTs[i], "amat": A16} for i in range(N_CORES)]
    res = bass_utils.run_bass_kernel_spmd(
        nc, in_maps, core_ids=list(range(N_CORES)),
        trace=trace, trace_cores=trace_cores,
    )
    nchunks = ROWS_PER_CORE // CHUNK
    T = CHUNK // P
    outs = []
    for r in res.results:
        o = np.asarray(r["out"])  # [nchunks, P, T*D] bf16
        o = o.reshape(nchunks, P, T, D).transpose(0, 2, 1, 3)
        outs.append(o.reshape(ROWS_PER_CORE, D))
    out = np.concatenate(outs, axis=0).astype(np.float32)
    return out, res


def kernel(x, W_dense, s_diag, U, V):
    A = _assemble_A(
        np.asarray(W_dense, dtype=np.float32),
        np.asarray(s_diag, dtype=np.float32),
        np.asarray(U, dtype=np.float32),
        np.asarray(V, dtype=np.float32),
    )
    out, _ = _run(np.asarray(x, dtype=np.float32), A)
    return out

